# revision 13
# baseline (speedup 1.0000x reference)
"""AuxSpatialGather (per-class masked mean pooling) Trainium2 kernel.

Computes, per sample b:  ctx[k, c] = mean over pixels n with gt[n]==k of feats[c, n]
(classes with zero pixels get 0), returned as [B, C, K, 1] float32.

Strategy (8 NeuronCores, data-parallel over batch, 2 samples/core):
  - feats arrive channel-major [C, HW]; the PE matmul contracts over the
    partition dim, so feats must become pixel-major on chip. fp32 matmul on
    TRN2 runs at ~1/4 rate, so feats are cast fp32->fp16 on DVE after plain
    f32 loads, then PE-transposed as PAIRS of fp16 pixels viewed as one f32
    element (halves the transpose count; PE transpose-mode is a bit-exact raw
    mover), evacuated PSUM->SBUF on ACT, and reduced by a one-hot matmul in
    fp16 (two parity-split matmuls over a stride-2 rhs view) with fp32 PSUM
    accumulation. Only precision loss: fp16 input quantization.
  - engine roles are strictly decoupled so the HBM stream never waits on
    compute: Sync (HWDGE q1) and GpSimd (SWDGE q0) alternate issuing feats
    loads and do nothing that waits on compute; DVE only casts (+ builds the
    one-hot planes); ACT does all PSUM evacuations and the mid-stream store
    (both wait on PE, which is fine - ACT issues no loads).
  - chunk schedule per sample: 7x2048px (1MiB DMAs per channel tile for
    stream efficiency) then 1024 + 2x512 px, so the end-of-stream tail is
    half a small chunk's PE work. Transpose windows are chunk-local.
  - the kernel emits RAW per-class sums [K, C]; per-class counts come from
    gt_seg_map on the host (exact integer bincount), which also does the
    mean division and the [K,C]->[C,K] transpose.
"""

import numpy as np

NUM_CLASSES = 19
B, C, H, W = 16, 512, 128, 128
HW = H * W
N_CORES = 8
S = B // N_CORES  # samples per core
P = 128  # partitions

_compiled = None


def _build_nc(s=S, c=C, hw=HW):
    from concourse import bacc, mybir
    from concourse.tile import TileContext
    from concourse.masks import make_identity

    f32 = mybir.dt.float32
    f16 = mybir.dt.float16
    i32 = mybir.dt.int32
    K = NUM_CLASSES
    n_ci = c // P  # channel tiles (4)
    n_t = hw // P  # 128-pixel weight columns per sample (128)
    # chunk schedule: tapered at BOTH ends. PE is slightly faster per byte
    # than the HBM stream but can only start a chunk once it has fully
    # landed+cast, so a big first chunk seeds a permanent "one big chunk
    # behind" lag that surfaces as dead PE time after the stream ends.
    # Small chunks first let PE start ~1us into the stream; small chunks
    # last make the post-stream tail one tiny chunk's work; 2048-px (1MiB
    # per ci DMA) chunks in the middle keep the stream efficient.
    sizes = [256, 256, 512, 1024] + [2048] * 6 + [1024, 512, 256, 256]
    assert sum(sizes) == hw
    chunk_specs = []
    off = 0
    for qw in sizes:
        chunk_specs.append((off, qw))
        off += qw
    n_q = len(chunk_specs)
    # consecutive same-size runs, for the gt load patterns
    gt_groups = []
    for px_base, qw in chunk_specs:
        if gt_groups and gt_groups[-1][1] == qw:
            gt_groups[-1][2] += 1
        else:
            gt_groups.append([px_base, qw, 1])

    nc = bacc.Bacc("TRN2", target_bir_lowering=False)
    feats = nc.dram_tensor("feats", [s, c, hw], f32, kind="ExternalInput")
    gt = nc.dram_tensor("gt_seg_map", [s, hw], i32, kind="ExternalInput")
    out = nc.dram_tensor("out", [s, K, c], f32, kind="ExternalOutput")

    with TileContext(nc) as tc:
        with (
            tc.tile_pool(name="const", bufs=1) as const_pool,
            tc.tile_pool(name="stage", bufs=12) as stage_pool,
            tc.tile_pool(name="chunks", bufs=3) as chunk_pool,
            tc.tile_pool(name="planes", bufs=2) as plane_pool,
            tc.tile_pool(name="ft", bufs=4) as ft_pool,
            tc.tile_pool(name="small", bufs=2) as small_pool,
            tc.tile_pool(name="ftp", bufs=6, space="PSUM") as ftp_pool,
            tc.tile_pool(name="accp", bufs=2, space="PSUM") as acc_pool,
        ):
            ident32 = const_pool.tile([P, P], f32)
            make_identity(nc, ident32[:])

            # Pixel order: n = px_base + (2*n_j)*p + 2*j + par
            # -> G[p, t], t = px_base//128 + 2j + par: per-partition runs of
            # 2*n_j contiguous gt elements; transpose windows are stride-n_j
            # pair columns local to a chunk.

            chunk_counter = [0]

            def load_chunk(si, px_base, qw):
                """f32 loads + DVE casts for one chunk. All feats loads ride
                the sync HWDGE ring: the Sync engine executes nothing that
                waits on compute, so the ring never starves (SWDGE bulk
                streaming measured ~17% lower per-SDMA-engine efficiency)."""
                eng = nc.sync
                chunk_counter[0] += 1
                chs = []
                for ci in range(n_ci):
                    st = stage_pool.tile([P, 2048], f32, name="st")
                    ch = chunk_pool.tile([P, 2048], f16, name=f"ch{ci}")
                    eng.dma_start(
                        out=st[:, :qw],
                        in_=feats[
                            si,
                            ci * P : (ci + 1) * P,
                            px_base : px_base + qw,
                        ],
                    )
                    nc.vector.tensor_copy(ch[:, :qw], st[:, :qw])
                    chs.append((ch, qw))
                return chs

            def build_planes(si):
                """One-hot planes for sample si (pair-order pixel layout).
                gt DMA via SWDGE: its tiny strided descriptors stay off the
                HWDGE ring."""
                G_i = plane_pool.tile([P, n_t], i32, name="G_i")
                for px_base, qw, n_run in gt_groups:
                    ca = px_base // P
                    cb = ca + n_run * (qw // P)
                    nc.gpsimd.dma_start(
                        out=G_i[:, ca:cb].rearrange("p (q r) -> p q r", q=n_run),
                        in_=gt[
                            si, px_base : px_base + n_run * qw
                        ].rearrange("(q p r) -> p q r", q=n_run, p=P),
                    )
                G_f = plane_pool.tile([P, n_t], f16, name="G_f")
                nc.vector.tensor_copy(G_f[:], G_i[:])
                planes = plane_pool.tile([P, K * n_t], f16, name="planes")
                for k in range(K):
                    nc.vector.tensor_scalar(
                        planes[:, k * n_t : (k + 1) * n_t],
                        G_f[:],
                        float(k),
                        None,
                        op0=mybir.AluOpType.is_equal,
                    )
                return planes

            # gt+planes first (SWDGE), then the first chunk
            planes_cur = build_planes(0)
            pending = load_chunk(0, *chunk_specs[0])

            # ---- main loop: load -> cast -> pair-transpose -> matmul ----
            for si in range(s):
                acc = acc_pool.tile([K, c], f32, name="acc")
                W_all = planes_cur[:].rearrange("p (k t) -> p t k", t=n_t)
                for q in range(n_q):
                    px_base, qw = chunk_specs[q]
                    t_base = px_base // P
                    n_j = qw // 256  # pair-windows (256 px) in this chunk
                    chs = pending
                    if q + 1 < n_q:
                        pending = load_chunk(si, *chunk_specs[q + 1])
                    elif si + 1 < s:
                        pending = load_chunk(si + 1, *chunk_specs[0])
                    if q == 5 and si + 1 < s:
                        planes_next = build_planes(si + 1)
                    # groups of <=4 windows; ci-major transposes within a
                    # group so PE needs only chunk ci0 to start the group
                    for g in range(0, n_j, 4):
                        gjs = range(g, min(n_j, g + 4))
                        ftps = {
                            jj: ftp_pool.tile([P, c], f32, name=f"ftp{jj % 4}", tag="ftp")
                            for jj in gjs
                        }
                        for ci in range(n_ci):
                            ch, cqw = chs[ci]
                            for jj in gjs:
                                nc.tensor.transpose(
                                    ftps[jj][:, ci * P : (ci + 1) * P],
                                    ch[:].bitcast(f32)[
                                        :, jj : jj + (P - 1) * n_j + 1 : n_j
                                    ],
                                    ident32[:],
                                )
                        for jj in gjs:
                            fts = ft_pool.tile([P, 2 * c], f16, name="fts")
                            nc.scalar.copy(fts[:].bitcast(f32), ftps[jj][:])
                            fts_pairs = fts[:].rearrange(
                                "p (c two) -> p two c", two=2
                            )
                            for par in range(2):
                                t = t_base + 2 * jj + par
                                nc.tensor.matmul(
                                    acc[:],
                                    W_all[:, t, :],
                                    fts_pairs[:, par, :],
                                    start=(t == 0),
                                    stop=(t == n_t - 1),
                                )

                # ---- emit raw sums [K, c] (PSUM -> SBUF on ACT -> DRAM) ----
                # mid-stream store rides ACT (already compute-coupled, issues
                # no loads); the final store rides the sync ring, after which
                # nothing else is queued there.
                acc_sb = small_pool.tile([K, c], f32, name="acc_sb")
                nc.scalar.copy(acc_sb[:], acc[:])
                store_eng = nc.gpsimd if si + 1 < s else nc.sync
                store_eng.dma_start(out=out[si], in_=acc_sb[:])
                if si + 1 < s:
                    planes_cur = planes_next
    nc.compile()
    return nc


def _get_compiled():
    global _compiled
    if _compiled is None:
        _compiled = _build_nc()
    return _compiled


def kernel(feats, gt_seg_map):
    from concourse.bass_utils import run_bass_kernel_spmd

    feats = np.asarray(feats, dtype=np.float32).reshape(B, C, HW)
    gt = np.asarray(gt_seg_map).reshape(B, HW)
    gt32 = gt.astype(np.int32)

    nc = _get_compiled()
    in_maps = []
    for i in range(N_CORES):
        in_maps.append(
            {
                "feats": feats[i * S : (i + 1) * S],
                "gt_seg_map": gt32[i * S : (i + 1) * S],
            }
        )
    res = run_bass_kernel_spmd(nc, in_maps, core_ids=list(range(N_CORES)))
    parts = [res.results[i]["out"] for i in range(N_CORES)]  # each [S, K, C]
    sums = np.concatenate(parts, axis=0).astype(np.float32)  # [B, K, C]

    # per-class pixel counts from gt (exact; mirrors the reference's
    # valid-mask + clip + one-hot sum)
    valid = gt != 255
    cl = np.clip(gt, 0, NUM_CLASSES - 1)
    onehot = (cl[:, None, :] == np.arange(NUM_CLASSES)[None, :, None]) & valid[:, None, :]
    cnt = onehot.sum(axis=2).astype(np.float32)  # [B, K]
    ctx = sums / np.maximum(cnt, 1.0)[:, :, None]  # [B, K, C]
    return np.transpose(ctx, (0, 2, 1))[..., None].astype(np.float32)  # [B, C, K, 1]
